# revision 15
# baseline (speedup 1.0000x reference)
"""Trainium2 Bass kernel for nn_LocalizationLoss.

Loss (see reference):
  p = out[:,:,0]; t = tgt[:,:,0] in {0,1}; mask = t
  bce  = -mean(t*ln(p) + (1-t)*ln(1-p))
  trick= out * t[...,None]
  CE over slot axis (dim 1) of trick[:,:,4:7] with targets tgt[:,:,4]
  Lx   = mean((trick_x - tx)^2), Ly likewise
  Lwh  = mean((t*sqrt(ow) - sqrt(tw))^2)
  loss = 5*(Lx+Ly+2*Lwh) + bce + 0.5*(1-bce) + 3*ce

Device computes, per core (batch-sharded), per-partition partial sums:
  S_bce  = sum ln(q),           q  = (2t-1)*(p-0.5)+0.5 = t?p:1-p
  S_sqxy = sum (t*ox-tx)^2 + (t*oy-ty)^2
  S_mw   = sum t*ow
  S_tw   = sum tw
  S_ts2  = sum t*2*sqrt(ow*tw)      [sqrt via exp(0.5*ln(m)+ln2), one ACT set]
  S_lse  = sum_j ln sum_i exp(t_i*o_i[4+j])
  S_sel  = sum_j sum_i eq(tgt_j,i) * t_i*o_i[4+j]
Host: Swh = S_mw + S_tw - S_ts2  (since (t*sqrt(ow)-sqrt(tw))^2
      == t*ow - 2*t*sqrt(ow*tw) + tw for t in {0,1})
      loss = 0.5 + (5*S_sqxy + 10*Swh - 0.5*S_bce + 3*(S_lse-S_sel)) / (3B)
"""

import numpy as np

import concourse.bass as bass
import concourse.bacc as bacc
import concourse.mybir as mybir
from concourse.tile import TileContext
from concourse.bass_utils import run_bass_kernel_spmd

# Force the ACT table pass to use only natural_log_exp_and_others (it holds
# every func this kernel needs: ln/exp/relu/abs/sign/square/copy/identity).
# The default greedy per-func set choice thrashes between sets, costing a
# ~1.3us ACT_TABLE_LOAD each time. Blank the other sets, keep dict order so
# act_func_set_id indices stay aligned with act_info.json.
import concourse.hw_specs as _hw_specs
if not hasattr(_hw_specs, "_orig_get_activation_tables"):
    _hw_specs._orig_get_activation_tables = _hw_specs.get_activation_tables

    def _only_ln_exp_tables(module_arch):
        tabs = _hw_specs._orig_get_activation_tables(module_arch)
        return {
            name: (funcs if name == "natural_log_exp_and_others" else set())
            for name, funcs in tabs.items()
        }

    _hw_specs.get_activation_tables = _only_ln_exp_tables
    # bacc imported the symbol at module load; patch its reference too
    import concourse.bacc as _bacc_mod
    if hasattr(_bacc_mod, "get_activation_tables"):
        _bacc_mod.get_activation_tables = _only_ln_exp_tables

F32 = mybir.dt.float32
ALU = mybir.AluOpType
ACT = mybir.ActivationFunctionType
LN2 = 0.6931471805599453

P = 128          # SBUF partitions
N_CORES = 8
B_FULL = 1_048_576

# per-chunk partial-sum column layout (MWTW = sum t*ow + tw combined)
COL_BCE, COL_SQXY, COL_MWTW, COL_TS2, COL_LSE, COL_SEL = range(6)
NCOL_PER_CHUNK = 6


def build_kernel(nb: int, n_chunks: int) -> bass.Bass:
    """Build the per-core Bass program for nb batch elements (ROWS=nb*3)."""
    rows = nb * 3
    assert rows % P == 0
    rpp = rows // P                 # rows per partition
    assert rpp % n_chunks == 0
    R = rpp // n_chunks             # rows per partition per chunk
    assert R % 3 == 0
    G = R // 3                      # b-groups per partition per chunk
    ncols = NCOL_PER_CHUNK * n_chunks

    nc = bacc.Bacc()

    # Register const [128,1] APs for activation bias values (non-Copy funcs
    # require an AP bias; only 0.0/1.0 are pre-registered by Bass.__init__).
    for val in (-0.5, 0.5, LN2, -1.0):
        ctile = nc.alloc_sbuf_tensor(f"const-f32-{val}", [128, 1], F32)
        nc.gpsimd.memset(ctile.ap(), val)
        nc.const_aps.aps[(F32, val)] = ctile.ap()
    nc.all_engine_barrier()

    out_hbm = nc.declare_dram_parameter("output", [rows * 7], F32, isOutput=False)
    tgt_hbm = nc.declare_dram_parameter("target", [rows * 5], F32, isOutput=False)
    res_hbm = nc.declare_dram_parameter("res", [P, ncols], F32, isOutput=True)

    out_v = out_hbm[:].rearrange("(p n) -> p n", p=P)   # [128, rpp*7]
    tgt_v = tgt_hbm[:].rearrange("(p n) -> p n", p=P)   # [128, rpp*5]

    with TileContext(nc) as tc:
        with (
            tc.tile_pool(name="io", bufs=2) as io_pool,
            tc.tile_pool(name="mid", bufs=2) as mid_pool,
            tc.tile_pool(name="accp", bufs=1) as acc_pool,
        ):
            cols = acc_pool.tile([P, ncols], F32)
            for c in range(n_chunks):
                cb = c * NCOL_PER_CHUNK

                ot = io_pool.tile([P, R * 7], F32, tag="ot")
                tt = io_pool.tile([P, R * 5], F32, tag="tt")
                nc.sync.dma_start(out=ot[:, :], in_=out_v[:, c * R * 7:(c + 1) * R * 7])
                nc.sync.dma_start(out=tt[:, :], in_=tgt_v[:, c * R * 5:(c + 1) * R * 5])

                o3 = ot[:, :].rearrange("p (r c) -> p r c", c=7)    # [128,R,7]
                t5 = tt[:, :].rearrange("p (r c) -> p r c", c=5)    # [128,R,5]
                t15 = tt[:, :].rearrange("p (g w) -> p g w", w=15)  # [128,G,15]

                p_ch = o3[:, :, 0]
                ow = o3[:, :, 3]
                t_ch = t5[:, :, 0]
                tw = t5[:, :, 3]
                tgt3 = t15[:, :, 4:15:5]                            # [128,G,3]

                # ---- scratch tiles (per chunk) ----
                M = mid_pool.tile([P, R * 6], F32, tag="M")      # masked o[1:7]
                E = mid_pool.tile([P, R * 3], F32, tag="E")      # exp(logits)
                EQ = mid_pool.tile([P, R * 3], F32, tag="EQ")    # one-hot eq0|eq1|eq2
                S = mid_pool.tile([P, R], F32, tag="S")          # sum_i exp(L)
                qp = mid_pool.tile([P, R], F32, tag="qp")
                exy = mid_pool.tile([P, R * 2], F32, tag="exy")
                m = mid_pool.tile([P, R], F32, tag="m")
                t2 = mid_pool.tile([P, R], F32, tag="t2")
                junka = mid_pool.tile([P, R], F32, tag="junka")  # ACT dump
                junkv = mid_pool.tile([P, R * 3], F32, tag="junkv")  # DVE dump

                M3 = M[:, :].rearrange("p (r k) -> p r k", k=6)     # [128,R,6]
                # logits live at M3[:, row, 3:6] with row = 3g+i (slot i of group g)
                L_rj = M3[:, :, 3:6]                                # [128,R,3]
                E4 = E[:, :].rearrange("p (g i j) -> p g i j", i=3, j=3)
                Ek = E[:, :].rearrange("p (k j) -> p k j", j=3)     # [128,R,3]
                EQ4 = EQ[:, :].rearrange("p (g i j) -> p g i j", i=3, j=3)
                EQk = EQ[:, :].rearrange("p (k j) -> p k j", j=3)   # [128,R,3]
                S_gj = S[:, :].rearrange("p (g j) -> p g j", j=3)
                junkv_k = junkv[:, :].rearrange("p (k j) -> p k j", j=3)

                # ---- prep on GPSIMD (otherwise idle): t2 = 2t-1 ----
                nc.gpsimd.tensor_scalar(t2[:, :], t_ch, 2.0, -1.0, ALU.mult, ALU.add)

                # ---- V1: M = o[1:7] * t (broadcast) ----
                t_b = t5[:, :, 0:1].broadcast_to([P, R, 6])
                nc.vector.scalar_tensor_tensor(M3, o3[:, :, 1:7], 0.0, t_b, ALU.bypass, ALU.mult)

                # ---- BCE: q' = (p-0.5)*t2 ; col += sum ln(q'+0.5) ----
                nc.vector.scalar_tensor_tensor(
                    qp[:, :], p_ch, 0.5, t2[:, :], ALU.subtract, ALU.mult
                )
                nc.scalar.activation(
                    qp[:, :], qp[:, :], ACT.Ln, bias=0.5, scale=1.0,
                    accum_out=cols[:, cb + COL_BCE:cb + COL_BCE + 1],
                )

                # ---- x/y MSE: exy = M[:, :, 0:2] - t[:, :, 1:3]; sum of squares ----
                exy2 = exy[:, :].rearrange("p (r k) -> p r k", k=2)
                nc.vector.scalar_tensor_tensor(exy2, M3[:, :, 0:2], 0.0, t5[:, :, 1:3], ALU.bypass, ALU.subtract)
                nc.scalar.activation(
                    exy[:, :], exy[:, :], ACT.Square,
                    accum_out=cols[:, cb + COL_SQXY:cb + COL_SQXY + 1],
                )

                # ---- wh: m = ow*tw; s2 = 2*sqrt(m) = exp(0.5*ln(m)+ln2) ----
                nc.vector.scalar_tensor_tensor(m[:, :], ow, 0.0, tw, ALU.bypass, ALU.mult)
                nc.scalar.activation(m[:, :], m[:, :], ACT.Ln)
                nc.scalar.activation(m[:, :], m[:, :], ACT.Exp, bias=LN2, scale=0.5)
                # col += sum t * s2
                nc.vector.scalar_tensor_tensor(
                    junkv[:, 0:R], m[:, :], 0.0, t_ch, ALU.bypass, ALU.mult,
                    accum_out=cols[:, cb + COL_TS2:cb + COL_TS2 + 1],
                )
                # col += sum (t*ow + tw)  (masked M channel 2 plus raw tw)
                nc.vector.scalar_tensor_tensor(
                    junkv[:, R:2 * R], M3[:, :, 2], 0.0, tw, ALU.bypass, ALU.add,
                    accum_out=cols[:, cb + COL_MWTW:cb + COL_MWTW + 1],
                )

                # ---- CE: E = exp(L) (dense, row order); S_j = sum_i E ----
                nc.scalar.activation(Ek, L_rj, ACT.Exp)
                nc.vector.scalar_tensor_tensor(
                    S_gj, E4[:, :, 0, :], 0.0, E4[:, :, 1, :], ALU.bypass, ALU.add
                )
                nc.vector.scalar_tensor_tensor(
                    S_gj, S_gj, 0.0, E4[:, :, 2, :], ALU.bypass, ALU.add
                )
                nc.scalar.activation(
                    S[:, :], S[:, :], ACT.Ln,
                    accum_out=cols[:, cb + COL_LSE:cb + COL_LSE + 1],
                )

                # ---- CE select: one-hot of tgt in {0,1,2} on GPSIMD ----
                nc.gpsimd.tensor_scalar(EQ4[:, :, 0, :], tgt3, 0.0, None, ALU.is_equal)
                nc.gpsimd.tensor_scalar(EQ4[:, :, 1, :], tgt3, 1.0, None, ALU.is_equal)
                nc.gpsimd.tensor_scalar(EQ4[:, :, 2, :], tgt3, 2.0, None, ALU.is_equal)
                nc.vector.scalar_tensor_tensor(
                    junkv_k, EQk, 0.0, L_rj, ALU.bypass, ALU.mult,
                    accum_out=cols[:, cb + COL_SEL:cb + COL_SEL + 1],
                )

            nc.sync.dma_start(out=res_hbm[:, :], in_=cols[:, :])

    nc.compile()
    return nc


def combine_results(res_list, n_chunks: int, b_total: int) -> np.float32:
    """Host-side combine of per-core [128, ncols] partial sums."""
    acc = np.zeros(NCOL_PER_CHUNK, dtype=np.float64)
    for res in res_list:
        r = res.astype(np.float64).reshape(P, n_chunks, NCOL_PER_CHUNK)
        acc += r.sum(axis=(0, 1))
    s_bce = acc[COL_BCE]
    s_sqxy = acc[COL_SQXY]
    s_wh = acc[COL_MWTW] - acc[COL_TS2]
    s_ce = acc[COL_LSE] - acc[COL_SEL]
    denom = 3.0 * b_total
    loss = 0.5 + (5.0 * s_sqxy + 10.0 * s_wh - 0.5 * s_bce + 3.0 * s_ce) / denom
    return np.float32(loss)


_CACHED = {}


def _get_nc(nb: int, n_chunks: int) -> bass.Bass:
    key = (nb, n_chunks)
    if key not in _CACHED:
        _CACHED[key] = build_kernel(nb, n_chunks)
    return _CACHED[key]


def run_on_cores(output: np.ndarray, target: np.ndarray, n_chunks: int = 4,
                 trace: bool = False):
    """Shard along batch, run on 8 cores, return (res_list, BassKernelResults)."""
    b = output.shape[0]
    nb = b // N_CORES
    nc = _get_nc(nb, n_chunks)
    in_maps = []
    for k in range(N_CORES):
        o = np.ascontiguousarray(output[k * nb:(k + 1) * nb]).reshape(-1)
        t = np.ascontiguousarray(target[k * nb:(k + 1) * nb]).reshape(-1)
        in_maps.append({"output": o, "target": t})
    results = run_bass_kernel_spmd(
        nc, in_maps, core_ids=list(range(N_CORES)), trace=trace
    )
    res_list = [r["res"] for r in results.results]
    return res_list, results


def kernel(output: np.ndarray, target: np.ndarray) -> np.ndarray:
    output = np.asarray(output, dtype=np.float32)
    target = np.asarray(target, dtype=np.float32)
    b = output.shape[0]
    res_list, _ = run_on_cores(output, target, n_chunks=4)
    return combine_results(res_list, n_chunks=4, b_total=b)


# revision 16
# speedup vs baseline: 1.7282x; 1.7282x over previous
"""Trainium2 Bass kernel for nn_LocalizationLoss.

Loss (see reference):
  p = out[:,:,0]; t = tgt[:,:,0] in {0,1}; mask = t
  bce  = -mean(t*ln(p) + (1-t)*ln(1-p))
  trick= out * t[...,None]
  CE over slot axis (dim 1) of trick[:,:,4:7] with targets tgt[:,:,4]
  Lx   = mean((trick_x - tx)^2), Ly likewise
  Lwh  = mean((t*sqrt(ow) - sqrt(tw))^2)
  loss = 5*(Lx+Ly+2*Lwh) + bce + 0.5*(1-bce) + 3*ce

Device computes, per core (batch-sharded), per-partition partial sums:
  S_bce  = sum ln(q),           q  = (2t-1)*(p-0.5)+0.5 = t?p:1-p
  S_sqxy = sum (t*ox-tx)^2 + (t*oy-ty)^2
  S_mwtw = sum (t*ow + tw)
  S_ts2  = sum t*2*sqrt(ow*tw)      [sqrt via exp(0.5*ln(m)+ln2), one ACT set]
  S_lse  = sum_j ln sum_i exp(t_i*o_i[4+j])
  S_seli = sum_j (tgt_j==i) * t_i*o_i[4+j]   for i in 0,1,2
Host: Swh = S_mwtw - S_ts2  (since (t*sqrt(ow)-sqrt(tw))^2
      == t*ow - 2*t*sqrt(ow*tw) + tw for t in {0,1})
      ce*3B = S_lse - (S_sel0+S_sel1+S_sel2)
      loss = 0.5 + (5*S_sqxy + 10*Swh - 0.5*S_bce + 3*ce*3B) / (3B)
"""

import numpy as np

import concourse.bass as bass
import concourse.bacc as bacc
import concourse.mybir as mybir
from concourse.tile import TileContext
from concourse.bass_utils import run_bass_kernel_spmd

# Force the ACT table pass to use only natural_log_exp_and_others (it holds
# every func this kernel needs: ln/exp/square/copy/identity). The default
# greedy per-func set choice thrashes between sets, costing a ~1.3us
# ACT_TABLE_LOAD each time. Blank the other sets, keep dict order so
# act_func_set_id indices stay aligned with act_info.json.
import concourse.hw_specs as _hw_specs
if not hasattr(_hw_specs, "_orig_get_activation_tables"):
    _hw_specs._orig_get_activation_tables = _hw_specs.get_activation_tables

    def _only_ln_exp_tables(module_arch):
        tabs = _hw_specs._orig_get_activation_tables(module_arch)
        return {
            name: (funcs if name == "natural_log_exp_and_others" else set())
            for name, funcs in tabs.items()
        }

    _hw_specs.get_activation_tables = _only_ln_exp_tables
    import concourse.bacc as _bacc_mod
    if hasattr(_bacc_mod, "get_activation_tables"):
        _bacc_mod.get_activation_tables = _only_ln_exp_tables

F32 = mybir.dt.float32
ALU = mybir.AluOpType
ACT = mybir.ActivationFunctionType
LN2 = 0.6931471805599453

P = 128          # SBUF partitions
N_CORES = 8
B_FULL = 1_048_576

# per-chunk partial-sum column layout
(COL_BCE, COL_SQXY, COL_MWTW, COL_TS2, COL_LSE,
 COL_SEL0, COL_SEL1, COL_SEL2) = range(8)
NCOL_PER_CHUNK = 8


def build_kernel(nb: int, n_chunks: int) -> bass.Bass:
    """Build the per-core Bass program for nb batch elements (ROWS=nb*3)."""
    rows = nb * 3
    assert rows % P == 0
    rpp = rows // P                 # rows per partition
    assert rpp % n_chunks == 0
    R = rpp // n_chunks             # rows per partition per chunk
    assert R % 3 == 0
    G = R // 3                      # b-groups per partition per chunk
    ncols = NCOL_PER_CHUNK * n_chunks

    nc = bacc.Bacc()

    # Register const [128,1] APs for activation bias values (non-Copy funcs
    # require an AP bias; only 0.0/1.0 are pre-registered by Bass.__init__).
    for val in (0.5, LN2):
        ctile = nc.alloc_sbuf_tensor(f"const-f32-{val}", [128, 1], F32)
        nc.gpsimd.memset(ctile.ap(), val)
        nc.const_aps.aps[(F32, val)] = ctile.ap()
    nc.all_engine_barrier()

    out_hbm = nc.declare_dram_parameter("output", [rows * 7], F32, isOutput=False)
    tgt_hbm = nc.declare_dram_parameter("target", [rows * 5], F32, isOutput=False)
    res_hbm = nc.declare_dram_parameter("res", [P, ncols], F32, isOutput=True)

    out_v = out_hbm[:].rearrange("(p n) -> p n", p=P)   # [128, rpp*7]
    tgt_v = tgt_hbm[:].rearrange("(p n) -> p n", p=P)   # [128, rpp*5]

    with TileContext(nc) as tc:
        with (
            tc.tile_pool(name="io", bufs=2) as io_pool,
            tc.tile_pool(name="mid", bufs=2) as mid_pool,
            tc.tile_pool(name="accp", bufs=1) as acc_pool,
        ):
            cols = acc_pool.tile([P, ncols], F32)
            for c in range(n_chunks):
                cb = c * NCOL_PER_CHUNK

                ot = io_pool.tile([P, R * 7], F32, tag="ot")
                tt = io_pool.tile([P, R * 5], F32, tag="tt")
                nc.sync.dma_start(out=ot[:, :], in_=out_v[:, c * R * 7:(c + 1) * R * 7])
                nc.sync.dma_start(out=tt[:, :], in_=tgt_v[:, c * R * 5:(c + 1) * R * 5])

                o3 = ot[:, :].rearrange("p (r c) -> p r c", c=7)    # [128,R,7]
                t5 = tt[:, :].rearrange("p (r c) -> p r c", c=5)    # [128,R,5]
                t15 = tt[:, :].rearrange("p (g w) -> p g w", w=15)  # [128,G,15]

                p_ch = o3[:, :, 0]
                ow = o3[:, :, 3]
                t_ch = t5[:, :, 0]
                tw = t5[:, :, 3]
                tgt3 = t15[:, :, 4:15:5]                            # [128,G,3]

                # ---- scratch tiles (per chunk) ----
                # M is channel-PLANAR: plane c (c=0..5) holds masked o-channel
                # c+1 for all R rows, so plane reads/writes are dense.
                M = mid_pool.tile([P, R * 6], F32, tag="M")
                E = mid_pool.tile([P, R * 3], F32, tag="E")      # exp(L), j-planar
                S = mid_pool.tile([P, R], F32, tag="S")          # sum_i exp(L), (g,j)
                qp = mid_pool.tile([P, R], F32, tag="qp")
                exy = mid_pool.tile([P, R * 2], F32, tag="exy")  # planar x,y
                m = mid_pool.tile([P, R], F32, tag="m")
                t2 = mid_pool.tile([P, R], F32, tag="t2")
                junkv = mid_pool.tile([P, R], F32, tag="junkv")  # DVE dump

                Mpl = M[:, :].rearrange("p (c r) -> p c r", c=6)    # [128,6,R]
                # L[row k, j] = Mpl[:, 3+j, k]; iteration (j,k): dense runs
                L_jk = Mpl[:, 3:6, :]                               # [128,3,R]
                E_jk = E[:, :].rearrange("p (j k) -> p j k", j=3)   # [128,3,R]
                # E addend slices for S: S[g,j] = sum_i E[j-plane][3g+i]
                E_gji = E[:, :].rearrange("p (j g i) -> p g j i", j=3, i=3)
                S_gj = S[:, :].rearrange("p (g j) -> p g j", j=3)
                exy_pl = exy[:, :].rearrange("p (c r) -> p c r", c=2)

                # ---- t2 = 2t-1 (DVE tensor_scalar) ----
                nc.vector.tensor_scalar(t2[:, :], t_ch, 2.0, -1.0, ALU.mult, ALU.add)

                # ---- V1: M planar = o[1:7] * t (broadcast) ----
                # iterate (c outer, r inner): in o3 offset 7r+1+c, t bcast, out dense
                o_cr = ot[:, :].rearrange("p (r c) -> p c r", c=7)[:, 1:7, :]
                t_b = t5[:, :, 0:1].broadcast_to([P, R, 6]).rearrange("p r c -> p c r")
                nc.vector.scalar_tensor_tensor(Mpl, o_cr, 0.0, t_b, ALU.bypass, ALU.mult)

                # ---- BCE: q' = (p-0.5)*t2 ; col += sum ln(q'+0.5) ----
                nc.vector.scalar_tensor_tensor(
                    qp[:, :], p_ch, 0.5, t2[:, :], ALU.subtract, ALU.mult
                )
                nc.scalar.activation(
                    qp[:, :], qp[:, :], ACT.Ln, bias=0.5, scale=1.0,
                    accum_out=cols[:, cb + COL_BCE:cb + COL_BCE + 1],
                )

                # ---- x/y MSE: exy = Mxy - txy ; col += sum square ----
                t_xy = t5[:, :, 1:3].rearrange("p r c -> p c r")    # [128,2,R]
                nc.vector.scalar_tensor_tensor(
                    exy_pl, Mpl[:, 0:2, :], 0.0, t_xy, ALU.bypass, ALU.subtract
                )
                nc.scalar.activation(
                    exy[:, :], exy[:, :], ACT.Square,
                    accum_out=cols[:, cb + COL_SQXY:cb + COL_SQXY + 1],
                )

                # ---- wh: m = ow*tw; s2 = 2*sqrt(m) = exp(0.5*ln(m)+ln2) ----
                nc.vector.scalar_tensor_tensor(m[:, :], ow, 0.0, tw, ALU.bypass, ALU.mult)
                nc.scalar.activation(m[:, :], m[:, :], ACT.Ln)
                nc.scalar.activation(m[:, :], m[:, :], ACT.Exp, bias=LN2, scale=0.5)
                # col += sum t * s2
                nc.vector.scalar_tensor_tensor(
                    junkv[:, :], m[:, :], 0.0, t_ch, ALU.bypass, ALU.mult,
                    accum_out=cols[:, cb + COL_TS2:cb + COL_TS2 + 1],
                )
                # col += sum (t*ow + tw)   (masked M plane 2 plus raw tw)
                nc.vector.scalar_tensor_tensor(
                    junkv[:, :], Mpl[:, 2, :], 0.0, tw, ALU.bypass, ALU.add,
                    accum_out=cols[:, cb + COL_MWTW:cb + COL_MWTW + 1],
                )

                # ---- CE: E = exp(L) (j-planar, dense); S_j = sum_i E ----
                nc.scalar.activation(E_jk, L_jk, ACT.Exp)
                nc.vector.scalar_tensor_tensor(
                    S_gj, E_gji[:, :, :, 0], 0.0, E_gji[:, :, :, 1],
                    ALU.bypass, ALU.add
                )
                nc.vector.scalar_tensor_tensor(
                    S_gj, S_gj, 0.0, E_gji[:, :, :, 2], ALU.bypass, ALU.add
                )
                nc.scalar.activation(
                    S[:, :], S[:, :], ACT.Ln,
                    accum_out=cols[:, cb + COL_LSE:cb + COL_LSE + 1],
                )

                # ---- CE select: col_i += sum (tgt==i) * L[3g+i, j] ----
                # L[3g+i, j] = Mpl[:, 3+j, 3g+i]: iteration (g,j) -> [[3,G],[R,3]]
                Lsel = M[:, :].rearrange("p (c g i) -> p i g c", c=6, i=3)[:, :, :, 3:6]
                for i in range(3):
                    nc.vector.scalar_tensor_tensor(
                        junkv[:, :].rearrange("p (g j) -> p g j", j=3),
                        tgt3, float(i), Lsel[:, i], ALU.is_equal, ALU.mult,
                        accum_out=cols[:, cb + COL_SEL0 + i:cb + COL_SEL0 + i + 1],
                    )

            nc.sync.dma_start(out=res_hbm[:, :], in_=cols[:, :])

    nc.compile()
    return nc


def combine_results(res_list, n_chunks: int, b_total: int) -> np.float32:
    """Host-side combine of per-core [128, ncols] partial sums."""
    acc = np.zeros(NCOL_PER_CHUNK, dtype=np.float64)
    for res in res_list:
        r = np.asarray(res).astype(np.float64).reshape(P, n_chunks, NCOL_PER_CHUNK)
        acc += r.sum(axis=(0, 1))
    s_bce = acc[COL_BCE]
    s_sqxy = acc[COL_SQXY]
    s_wh = acc[COL_MWTW] - acc[COL_TS2]
    s_ce = acc[COL_LSE] - (acc[COL_SEL0] + acc[COL_SEL1] + acc[COL_SEL2])
    denom = 3.0 * b_total
    loss = 0.5 + (5.0 * s_sqxy + 10.0 * s_wh - 0.5 * s_bce + 3.0 * s_ce) / denom
    return np.float32(loss)


_CACHED = {}


def _get_nc(nb: int, n_chunks: int) -> bass.Bass:
    key = (nb, n_chunks)
    if key not in _CACHED:
        _CACHED[key] = build_kernel(nb, n_chunks)
    return _CACHED[key]


def run_on_cores(output: np.ndarray, target: np.ndarray, n_chunks: int = 4,
                 trace: bool = False):
    """Shard along batch, run on 8 cores, return (res_list, BassKernelResults)."""
    b = output.shape[0]
    nb = b // N_CORES
    nc = _get_nc(nb, n_chunks)
    in_maps = []
    for k in range(N_CORES):
        o = np.ascontiguousarray(output[k * nb:(k + 1) * nb]).reshape(-1)
        t = np.ascontiguousarray(target[k * nb:(k + 1) * nb]).reshape(-1)
        in_maps.append({"output": o, "target": t})
    results = run_bass_kernel_spmd(
        nc, in_maps, core_ids=list(range(N_CORES)), trace=trace
    )
    res_list = [r["res"] for r in results.results]
    return res_list, results


def kernel(output: np.ndarray, target: np.ndarray) -> np.ndarray:
    output = np.asarray(output, dtype=np.float32)
    target = np.asarray(target, dtype=np.float32)
    b = output.shape[0]
    res_list, _ = run_on_cores(output, target, n_chunks=4)
    return combine_results(res_list, n_chunks=4, b_total=b)
